# revision 1
# baseline (speedup 1.0000x reference)
"""Additive (Bahdanau) content attention on 8 Trainium2 NeuronCores.

  dec_proj = decoder_output @ W            [B,1,C]
  enc_proj = encoder_outputs @ V           [B,T,C]
  energy   = tanh(dec_proj + enc_proj + b) [B,T,C]
  scores   = energy @ w                    [B,T]
  align    = softmax(scores)               [B,T]
  context  = align @ encoder_outputs       [B,H]

Sharding: data-parallel over batch, 4 batch items per core, no collectives.
The encoder is pre-transposed on the host to [B, H, T] so the contraction
dim (H) sits on SBUF partitions; the big matmul runs as
projT[c,t] = V[h,c]^T @ encT[h,t] in float32r (full PE rate).
ACT fuses the (dec_proj+bias) add into tanh via its per-partition bias
operand.  Scores fold w over C with M=1 PE matmuls accumulating in PSUM.
Softmax runs unnormalized flash-style per T-half (scores are bounded by
sum|w| ~ 26, so exp never overflows fp32); the context accumulates with
DVE tensor_tensor_reduce and is scaled by 1/sum(exp) once at the end.
"""

import numpy as np

B, T, H, C = 32, 2048, 1024, 1024
N_CORES = 8
B_LOC = B // N_CORES          # 4 batch items per core
T_HALF = 1024                 # T streamed in halves per batch item
N_HALVES = T // T_HALF        # 2
KC = H // 128                 # 8 contraction chunks (h)
CC = C // 128                 # 8 context-size chunks (c)
HC = H // 128                 # 8 output chunks (h)

_COMPILED = {}


def _split_excess_waits(nc, mybir):
    """Pinned-walrus workaround: an instruction may carry at most 1 sem wait
    (2 for EventSemaphore).  Tile's end-of-kernel drain violates this; hoist
    excess waits onto inserted Drain instructions on the same engine."""
    for func in nc.m.functions:
        for bb in func.blocks:
            insts = bb.instructions
            i = 0
            while i < len(insts):
                inst = insts[i]
                si = inst.sync_info
                if si is not None:
                    waits = list(si.on_wait)
                    cap = 2 if type(inst).__name__ == "InstEventSemaphore" else 1
                    if len(waits) > cap:
                        carriers = []
                        for w in waits[: len(waits) - cap]:
                            d = mybir.InstDrain(
                                name=nc.get_next_instruction_name(),
                                ins=[],
                                outs=[],
                                bass_is_fusable=False,
                            )
                            d.engine = inst.engine
                            d.sync_info = mybir.SyncInfo(on_wait=[w], on_update=[])
                            carriers.append(d)
                        si.on_wait = waits[len(waits) - cap :]
                        for k, d in enumerate(carriers):
                            insts.insert(i + k, d)
                        i += len(carriers)
                i += 1


def _build(mm_dtype="float32r"):
    import concourse.bass as bass
    import concourse.tile as tile
    import concourse.mybir as mybir

    dt = mybir.dt
    F32 = dt.float32
    MMD = getattr(dt, mm_dtype)  # matmul operand dtype (float32r or float32)
    AF = mybir.ActivationFunctionType
    ALU = mybir.AluOpType

    nc = bass.Bass("TRN2", target_bir_lowering=False, debug=False)
    encT = nc.dram_tensor("encT", [B_LOC, H, T], F32, kind="ExternalInput").ap()
    # consts[:, 0:8]=bias  [:, 8:16]=w  [:, 16:48]=decT (col = k*B_LOC + b)
    constsd = nc.dram_tensor("consts", [128, 16 + KC * B_LOC], F32,
                             kind="ExternalInput").ap()
    # W/V pre-shuffled on host to c-chunk-major: [c][p][k*128+j] so each
    # c-chunk is one contiguous [128, KC*128] block with 4KB partition lines
    Wd = nc.dram_tensor("W", [CC, 128, KC * 128], F32, kind="ExternalInput").ap()
    Vd = nc.dram_tensor("V", [CC, 128, KC * 128], F32, kind="ExternalInput").ap()
    ctxd = nc.dram_tensor("ctx", [B_LOC, H], F32, kind="ExternalOutput").ap()

    with tile.TileContext(nc) as tc:
        with (
            tc.tile_pool(name="const", bufs=1) as constp,
            tc.tile_pool(name="slab", bufs=2) as slab_p,
            tc.tile_pool(name="slabf", bufs=1) as slabf_p,
            tc.tile_pool(name="energy", bufs=3) as energy_p,
            tc.tile_pool(name="alpha", bufs=2) as alpha_p,
            tc.tile_pool(name="scratch", bufs=1) as scratch_p,
            tc.tile_pool(name="small", bufs=4) as small_p,
            tc.tile_pool(name="ctxp", bufs=4) as ctx_p,
        ):
            # ---------- prefetch the first encoder slab before anything ----------
            # (split across all three DMA paths so it lands ~25% sooner)
            slab0 = slab_p.tile([128, KC * T_HALF], MMD, tag="slab", name="slab0")
            for k in range(KC - 2):
                nc.gpsimd.dma_start(
                    slab0[:, k * T_HALF : (k + 1) * T_HALF],
                    encT[0, k * 128 : (k + 1) * 128, 0:T_HALF],
                )
            with tc.tile_pool(name="sstg", bufs=2) as sstg_p:
                for j, k in enumerate(range(KC - 2, KC)):
                    sstg = sstg_p.tile([128, T_HALF], F32, tag="ss", name=f"sstg{k}")
                    eng = nc.sync if j == 0 else nc.scalar
                    eng.dma_start(sstg[:], encT[0, k * 128 : (k + 1) * 128, 0:T_HALF])
                    nc.vector.tensor_copy(
                        slab0[:, k * T_HALF : (k + 1) * T_HALF], sstg[:])

            # ---------- constants (one descriptor-efficient DMA) ----------
            consts_sb = constp.tile([128, 16 + KC * B_LOC], F32)
            nc.sync.dma_start(consts_sb[:], constsd[:])
            bias_sb = consts_sb[:, 0:CC]
            w_sb = consts_sb[:, CC : 2 * CC]
            ones_f = constp.tile([1, 128], F32)
            nc.vector.memset(ones_f[:], 1.0)
            ones_r = constp.tile([1, 128], MMD)
            nc.vector.tensor_copy(ones_r[:], ones_f[:])
            w_sbr = constp.tile([128, CC], MMD)
            nc.vector.tensor_copy(w_sbr[:], w_sb)
            decT_sb = constp.tile([128, KC * B_LOC], MMD)
            nc.vector.tensor_copy(decT_sb[:], consts_sb[:, 16 : 16 + KC * B_LOC])
            # V/W are loaded BY C-CHUNK, just in time with the first batch's
            # c-loop, on the two HWDGE queues (V on ACT, W on SP) as fp32,
            # then DVE-encoded to the matmul dtype.  dec_proj is likewise
            # computed per c-chunk inline.  This keeps the SWDGE cast path
            # free for encoder slabs and removes the serial DMA head.
            v_sb = constp.tile([128, CC * KC * 128], MMD)  # (c,k)-major
            dpb_sb = constp.tile([128, CC * B_LOC], F32)

            # ---------- main pipeline ----------
            with (
                tc.tile_pool(name="stage", bufs=2) as stage_p,
                tc.tile_pool(name="wenc", bufs=2) as wenc_p,
                tc.tile_pool(name="ps_proj", bufs=3, space="PSUM") as ps_proj,
                tc.tile_pool(name="ps_sc", bufs=1, space="PSUM") as ps_sc,
                tc.tile_pool(name="ps_b", bufs=1, space="PSUM") as ps_b,
                tc.tile_pool(name="ps_dp", bufs=1, space="PSUM") as ps_dp,
            ):
                for b in range(B_LOC):
                    asum = small_p.tile([1, 2 * N_HALVES], F32, tag="asum")
                    ctx_halves = []
                    for half in range(N_HALVES):
                        # -- load encT slab [128, KC*T_HALF] for (b, half)
                        # (cast-DMA to the matmul dtype; decoded to fp32
                        # afterwards for the context reduce)
                        if b == 0 and half == 0:
                            slab = slab0
                        else:
                            slab = slab_p.tile([128, KC * T_HALF], MMD, tag="slab",
                                               name=f"slab{b}_{half}")
                            for k in range(KC):
                                nc.gpsimd.dma_start(
                                    slab[:, k * T_HALF : (k + 1) * T_HALF],
                                    encT[b, k * 128 : (k + 1) * 128,
                                         half * T_HALF : (half + 1) * T_HALF],
                                )

                        # -- projT + tanh + scores over c chunks
                        sc_ps = ps_sc.tile([1, T_HALF], F32, tag="sc")
                        pend = None  # delayed scores emission for PE slack
                        for c in range(CC):
                            if b == 0 and half == 0:
                                # JIT: load V/W c-chunk, encode, dec_proj + bias
                                vstg = stage_p.tile([128, KC * 128], F32,
                                                    tag="vstg", name=f"vstg{c}")
                                nc.scalar.dma_start(vstg[:], Vd[c])
                                nc.vector.tensor_copy(
                                    v_sb[:, c * KC * 128 : (c + 1) * KC * 128],
                                    vstg[:],
                                )
                                wstg = stage_p.tile([128, KC * 128], F32,
                                                    tag="wstg", name=f"wstg{c}")
                                nc.sync.dma_start(wstg[:], Wd[c])
                                wsl = wenc_p.tile([128, KC * 128], MMD,
                                                  tag="wsl", name=f"wsl{c}")
                                nc.vector.tensor_copy(wsl[:], wstg[:])
                                dp = ps_dp.tile([128, B_LOC], F32, tag="dp",
                                                name=f"dp{c}")
                                for k in range(KC):
                                    nc.tensor.matmul(
                                        dp[:],
                                        wsl[:, k * 128 : (k + 1) * 128],
                                        decT_sb[:, k * B_LOC : (k + 1) * B_LOC],
                                        start=(k == 0),
                                        stop=(k == KC - 1),
                                    )
                                nc.scalar.activation(
                                    dpb_sb[:, c * B_LOC : (c + 1) * B_LOC],
                                    dp[:],
                                    AF.Identity,
                                    bias=bias_sb[:, c : c + 1],
                                )
                            energy = energy_p.tile([128, T_HALF], MMD, tag="en")
                            projs = [
                                ps_proj.tile([128, 512], F32, tag="pj", name=f"pj{c}_{blk}")
                                for blk in range(T_HALF // 512)
                            ]
                            for k in range(KC):
                                for blk in range(T_HALF // 512):
                                    nc.tensor.matmul(
                                        projs[blk][:],
                                        v_sb[:, (c * KC + k) * 128 :
                                             (c * KC + k + 1) * 128],
                                        slab[:, k * T_HALF + blk * 512 :
                                             k * T_HALF + blk * 512 + 512],
                                        start=(k == 0),
                                        stop=(k == KC - 1),
                                    )
                            for blk in range(T_HALF // 512):
                                nc.scalar.activation(
                                    energy[:, blk * 512 : (blk + 1) * 512],
                                    projs[blk][:],
                                    AF.Tanh,
                                    bias=dpb_sb[:, c * B_LOC + b : c * B_LOC + b + 1],
                                )
                            if pend is not None:
                                pc, pen = pend
                                for blk in range(T_HALF // 512):
                                    nc.tensor.matmul(
                                        sc_ps[:, blk * 512 : (blk + 1) * 512],
                                        w_sbr[:, pc : pc + 1],
                                        pen[:, blk * 512 : (blk + 1) * 512],
                                        start=(pc == 0),
                                        stop=(pc == CC - 1),
                                    )
                            pend = (c, energy)
                        pc, pen = pend
                        for blk in range(T_HALF // 512):
                            nc.tensor.matmul(
                                sc_ps[:, blk * 512 : (blk + 1) * 512],
                                w_sbr[:, pc : pc + 1],
                                pen[:, blk * 512 : (blk + 1) * 512],
                                start=False,
                                stop=(pc == CC - 1),
                            )

                        # -- decode the slab to fp32 for the context reduce
                        #    (DVE can't read fp32r operands directly)
                        if MMD != F32:
                            slab_f = slabf_p.tile([128, KC * T_HALF], F32, tag="sf")
                            nc.vector.tensor_copy(slab_f[:], slab[:])
                        else:
                            slab_f = slab

                        # -- exp (unnormalized) + per-blk sums
                        alpha_u = alpha_p.tile([1, T_HALF], MMD, tag="au")
                        for blk in range(T_HALF // 512):
                            nc.scalar.activation(
                                alpha_u[:, blk * 512 : (blk + 1) * 512],
                                sc_ps[:, blk * 512 : (blk + 1) * 512],
                                AF.Exp,
                                accum_out=asum[:, half * 2 + blk : half * 2 + blk + 1],
                            )

                        # -- broadcast alpha_u across partitions (ones matmul)
                        ab_ps = ps_b.tile([128, T_HALF], F32, tag="ab")
                        for blk in range(T_HALF // 512):
                            nc.tensor.matmul(
                                ab_ps[:, blk * 512 : (blk + 1) * 512],
                                ones_r[:],
                                alpha_u[:, blk * 512 : (blk + 1) * 512],
                                start=True,
                                stop=True,
                            )
                        alpha_bs = alpha_p.tile([128, T_HALF], F32, tag="ab_sb")
                        for blk in range(T_HALF // 512):
                            nc.scalar.copy(
                                alpha_bs[:, blk * 512 : (blk + 1) * 512],
                                ab_ps[:, blk * 512 : (blk + 1) * 512],
                            )

                        # -- context accumulate: ctx[h] (+)= sum_t encT*alpha
                        ctx_cur = ctx_p.tile([128, HC], F32, tag="ctx")
                        for h in range(HC):
                            eng = nc.vector
                            scr = scratch_p.tile(
                                [128, T_HALF], F32, tag="scr", name=f"scr{h}")
                            eng.scalar_tensor_tensor(
                                out=scr[:],
                                in0=slab_f[:, h * T_HALF : (h + 1) * T_HALF],
                                scalar=1.0,
                                in1=alpha_bs[:],
                                op0=ALU.mult,
                                op1=ALU.mult,
                                accum_out=ctx_cur[:, h : h + 1],
                            )
                        ctx_halves.append(ctx_cur)

                    # -- normalize and store
                    ctx_sum = small_p.tile([128, HC], F32, tag="cs")
                    nc.vector.tensor_add(ctx_sum[:], ctx_halves[0][:], ctx_halves[1][:])
                    total = small_p.tile([1, 1], F32, tag="tot")
                    nc.vector.reduce_sum(total[:], asum[:], axis=mybir.AxisListType.X)
                    recip = small_p.tile([1, 1], F32, tag="rec")
                    nc.vector.reciprocal(recip[:], total[:])
                    rb_ps = ps_b.tile([128, 1], F32, tag="ab")
                    nc.tensor.matmul(rb_ps[:], ones_f[:], recip[:], start=True, stop=True)
                    recip_bs = small_p.tile([128, 1], F32, tag="rbs")
                    nc.scalar.copy(recip_bs[:], rb_ps[:])
                    ctx_fin = small_p.tile([128, HC], F32, tag="cf")
                    nc.vector.tensor_scalar_mul(ctx_fin[:], ctx_sum[:], recip_bs[:])
                    nc.sync.dma_start(
                        ctxd.rearrange("b (hc p) -> b p hc", p=128)[b],
                        ctx_fin[:],
                    )

    return nc


def _get_nc(mode):
    if mode not in _COMPILED:
        import concourse.mybir as mybir

        nc = _build(mode)
        _split_excess_waits(nc, mybir)  # HW-compile-only fixup (breaks CoreSim)
        _COMPILED[mode] = nc
    return _COMPILED[mode]


def _prep_in_maps(decoder_output, encoder_outputs, W, V, b, w):
    dec = np.asarray(decoder_output, dtype=np.float32)
    enc = np.asarray(encoder_outputs, dtype=np.float32)

    def shuffle_cmajor(M):
        # [H, C] -> [CC, 128(p), KC*128]: block (c) holds M[k*128+p, c*128+j]
        return np.ascontiguousarray(
            np.asarray(M, dtype=np.float32)
            .reshape(KC, 128, CC, 128).transpose(2, 1, 0, 3)
            .reshape(CC, 128, KC * 128))

    Wf = shuffle_cmajor(W)
    Vf = shuffle_cmajor(V)
    bias_cols = np.asarray(b, dtype=np.float32).reshape(CC, 128).T    # [128, CC]
    w_cols = np.asarray(w, dtype=np.float32)[:, 0].reshape(CC, 128).T  # [128, CC]

    in_maps = []
    for core in range(N_CORES):
        s = slice(core * B_LOC, (core + 1) * B_LOC)
        encT = np.ascontiguousarray(enc[s].transpose(0, 2, 1))        # [B_LOC,H,T]
        # decT cols: [128, KC*B_LOC], col k*B_LOC+b = dec[b, k*128+p]
        decT_cols = (
            dec[s, 0, :].T.reshape(KC, 128, B_LOC).transpose(1, 0, 2)
            .reshape(128, KC * B_LOC)
        )
        consts = np.ascontiguousarray(
            np.concatenate([bias_cols, w_cols, decT_cols], axis=1))   # [128, 48]
        in_maps.append({"encT": encT, "consts": consts, "W": Wf, "V": Vf})
    return in_maps


def kernel(decoder_output, encoder_outputs, W, V, b, w):
    import os
    from concourse.bass_utils import run_bass_kernel_spmd

    mode = os.environ.get("ATT_MM_DTYPE", "float32r")
    nc = _get_nc(mode)
    in_maps = _prep_in_maps(decoder_output, encoder_outputs, W, V, b, w)
    res = run_bass_kernel_spmd(nc, in_maps, core_ids=list(range(N_CORES)))
    return np.concatenate([res.results[i]["ctx"] for i in range(N_CORES)], axis=0)

